# revision 12
# baseline (speedup 1.0000x reference)
"""Trainium2 Bass kernel for nn_Attention_32049045963483 (sparse_attention).

Math collapse (validated vs reference at ~4e-4 rel err):
  - qkv 1x1 conv folds into the 11x11/stride-8 down-convs (host-side
    weight composition): w_eff[d, ky, kx, oc] = sum_ic wq[oc,ic,ky,kx] W1[ic,d]
  - nearest-neighbor 64x upsample + softmax == softmax of the low-res map;
    out row I depends only on x = I//64.
  - v enters only through 64-wide block sums: vbar = Wv @ fbar,
    fbar[d,x] = sum_y f[d,x,y] (v never materializes).
  - out[c,x,y] = (sum_j e[j,x] vbar[j,c]) / (64 sum_j e[j,x]), broadcast on y.

Kernel structure (one head per core, 8 cores):
  - DIRECT conv: 66 matmuls accumulate into one PSUM [16,64] tile
    (q oc 0..7, k oc 8..15).  K = 128 = (64 d) x (2 kx taps): partitions
    64..127 hold f shifted by one column, so kx pairs (0,1)..(8,9) use the
    full PE contraction; kx=10 has zero weights in the upper half.
  - f rows are CLASS-ORDERED (r mod 8) in DRAM so arriving chunks unlock
    their matmuls: class c serves ky = c (and ky = c+8 for c < 3).
  - conv stationaries are packed in CONSUMPTION order; the first 24
    (classes 0-1) split across both HWDGE rings ahead of f; the rest on
    the gpsimd SWDGE queue.
  - gelu via tanh-approx (exp_and_others act table has Tanh+Exp+Square:
    NO mid-kernel activation-table reload), fused to 4 ops:
      x = psqk+bias (DVE) || u = (psqk+bias)^2 (Act Square)
      v2 = (u + a/b)*x (DVE stt);  th = tanh(b*v2) (Act scale)
      qk = (th + 1)*x (DVE stt) = 2*gelu(x); the 4x folds into exp scale.
  - fbar per class on DVE (overlaps conv), vbar via one small matmul.
  - output: x-major broadcast then transposed DMA split over both rings.
"""

import numpy as np

N_CORES = 8
SCALE = 8.0 ** -0.5  # dim_head ** -0.5

ORDER = [r for c in range(8) for r in range(c, 67, 8)]
CLS_OFF = [0, 9, 18, 27, 35, 43, 51, 59]
# conv matmul emission order: (class, ky, kx-pair); stationary block j is
# packed at w columns 16j
MM_ARGS = []
for _c in range(8):
    for _ky in ([_c, _c + 8] if _c + 8 <= 10 else [_c]):
        _pos0 = 0 if _ky == _c else 1
        for _p in range(6):
            MM_ARGS.append((_ky, _p, CLS_OFF[_c] + _pos0, 2 * _p))
N_MM = len(MM_ARGS)  # 66
C_WVT = 16 * N_MM            # 1056
C_BIAS = C_WVT + 8           # 1064
C_SEL = C_BIAS + 1           # 1065
W_COLS = C_SEL + 8           # 1073
W1_COLS = 16 * 24            # stationaries for classes 0 and 1 (24 matmuls)

_CACHE = {}
LAST_RESULTS = None  # BassKernelResults of the most recent run (for test harness)


def _dep(after, before, sync=False):
    from concourse.tile import add_dep_helper

    a = getattr(after, "ins", after)
    b = getattr(before, "ins", before)
    add_dep_helper(a, b, sync=sync, reason="pin order")


def _build_nc():
    from contextlib import ExitStack

    import concourse.bacc as bacc
    import concourse.bass as bass
    import concourse.mybir as mybir
    import concourse.tile as tile

    f32 = mybir.dt.float32
    f16 = mybir.dt.float16
    X = mybir.AxisListType.X
    AF = mybir.ActivationFunctionType
    OP = mybir.AluOpType

    nc = bacc.Bacc("TRN2", target_bir_lowering=False)

    f_d = nc.dram_tensor("f", [128, 67 * 68], f16, kind="ExternalInput")
    w_d = nc.dram_tensor("w", [128, W_COLS], f16, kind="ExternalInput")
    out_d = nc.dram_tensor("out", [8, 4096], f32, kind="ExternalOutput")

    with tile.TileContext(nc) as tc:
        with ExitStack() as ctx:
            sb = ctx.enter_context(tc.tile_pool(name="sb", bufs=1))
            ps = ctx.enter_context(tc.tile_pool(name="ps", bufs=1, space="PSUM"))

            f_t = sb.tile([128, 67 * 68], f16)
            w_t = sb.tile([128, W_COLS], f16)
            x_t = sb.tile([16, 64], f32)
            u_t = sb.tile([16, 64], f32)
            v2_t = sb.tile([16, 64], f32)
            th_t = sb.tile([16, 64], f32)
            qk_t = sb.tile([16, 64], f16)
            k_t = sb.tile([8, 64], f16)
            e_t = sb.tile([64, 64], f16)
            fbar_t = sb.tile([64, 64], f16)
            vaug_t = sb.tile([64, 9], f16)
            bias_t = sb.tile([16, 1], f32)
            rs_t = sb.tile([64, 1], f32)
            olT_t = sb.tile([64, 8], f32)
            T_t = sb.tile([64, 8 * 64], f32)
            scr_t = sb.tile([1, 1], f32)
            scr2_t = sb.tile([1, 1], f32)

            # --- input DMAs: first conv stationaries (classes 0-1) split
            # over both HWDGE rings, then f classes; bulk weights on SWDGE.
            def fseg(eng, c0, c1):
                a = CLS_OFF[c0] * 68
                b = (CLS_OFF[c1] + (9 if c1 < 3 else 8)) * 68
                return eng.dma_start(out=f_t[:, a:b], in_=f_d[:, a:b])

            HW1 = W1_COLS // 2
            nc.sync.dma_start(out=w_t[:, 0:HW1], in_=w_d[:, 0:HW1])
            nc.scalar.dma_start(out=w_t[:, HW1:W1_COLS], in_=w_d[:, HW1:W1_COLS])
            fseg(nc.sync, 0, 0)
            fseg(nc.scalar, 1, 1)
            d_w2 = nc.gpsimd.dma_start(
                out=w_t[:, W1_COLS:W_COLS], in_=w_d[:, W1_COLS:W_COLS]
            )
            fseg(nc.sync, 2, 3)
            d_c45 = fseg(nc.scalar, 4, 5)
            fseg(nc.sync, 6, 7)

            # constants + act-table preload (exp_and_others: Exp/Tanh/Square/
            # Copy in one set -> single ACT_TABLE_LOAD, after scalar triggers)
            nc.vector.memset(scr_t, 0.0)
            nc.vector.memset(vaug_t[:, 8:9], 64.0)
            de = nc.scalar.activation(out=scr2_t, in_=scr_t, func=AF.Exp)
            _dep(de, d_c45)
            bcp = nc.vector.tensor_copy(out=bias_t, in_=w_t[0:16, C_BIAS:C_BIAS + 1])
            _dep(bcp, d_w2)

            # --- direct conv: 66 matmuls accumulate into psqk [16, 64]
            f3 = f_t.rearrange("p (r c) -> p r c", c=68)
            psqk = ps.tile([16, 64], f32, tag="A")
            for n, (_ky, _p, r0, kx0) in enumerate(MM_ARGS):
                nc.tensor.matmul(
                    psqk, w_t[:, 16 * n:16 * n + 16],
                    f3[:, r0:r0 + 8, kx0:kx0 + 57:8],
                    start=(n == 0), stop=(n == N_MM - 1),
                )

            # --- fbar per class on DVE (runs under the conv).  fp16 out is
            # fine: 64-term sums of O(1) values, 40x margin under the gate.
            with nc.allow_low_precision("fbar fp16 is within tolerance"):
                for c in range(8):
                    p0 = 1 if c <= 1 else 0
                    x0 = (c - 2) % 8
                    nc.vector.tensor_reduce(
                        out=fbar_t[:, x0:x0 + 57:8],
                        in_=f3[0:64, CLS_OFF[c] + p0:CLS_OFF[c] + p0 + 8, 2:66],
                        axis=X, op=OP.add,
                    )

            # --- vbar: psv[x, c] = sum_d fbar[d, x] wvt[d, c]  (during gelu)
            psv = ps.tile([64, 8], f32, tag="C")
            nc.tensor.matmul(
                psv, fbar_t, w_t[0:64, C_WVT:C_WVT + 8], start=True, stop=True
            )
            nc.vector.tensor_copy(out=vaug_t[:, 0:8], in_=psv)

            # --- gelu (tanh approx): qk = 2*gelu(psqk + bias)
            #   x = psqk+bias;  u = x^2;  v2 = (u + a/b)*x;  th = tanh(b*v2)
            #   qk = (th+1)*x
            A_GELU = 0.79788456
            B_GELU = 0.0356774
            nc.vector.tensor_scalar_add(x_t, psqk, bias_t)
            nc.scalar.activation(out=u_t, in_=psqk, func=AF.Square, bias=bias_t)
            nc.vector.scalar_tensor_tensor(
                v2_t, u_t, A_GELU / B_GELU, x_t, op0=OP.add, op1=OP.mult
            )
            nc.scalar.activation(out=th_t, in_=v2_t, func=AF.Tanh, scale=B_GELU)
            nc.vector.scalar_tensor_tensor(
                qk_t, th_t, 1.0, x_t, op0=OP.add, op1=OP.mult
            )

            # k half -> base-0 tile via PE selection matmul (engines cannot
            # read at a partition offset), then DVE copy PSUM -> SBUF
            psk = ps.tile([8, 64], f32, tag="C")
            nc.tensor.matmul(psk, w_t[0:16, C_SEL:C_SEL + 8], qk_t,
                             start=True, stop=True)
            nc.vector.tensor_copy(out=k_t, in_=psk)

            # --- dots^T[j, i] = sum_c k[c, j] q[c, i];  e = exp(scale/4 * .)
            psd = ps.tile([64, 64], f32, tag="B")
            nc.tensor.matmul(psd, k_t, qk_t[0:8, :], start=True, stop=True)
            nc.scalar.activation(out=e_t, in_=psd, func=AF.Exp, scale=SCALE / 4.0)

            # --- out_u[x, 0:8] = sum_j e[j, x] vbar[j, c]; col 8 = 64*sum_j e
            pso = ps.tile([64, 9], f32, tag="D")
            nc.tensor.matmul(pso, e_t, vaug_t, start=True, stop=True)
            nc.vector.reciprocal(out=rs_t, in_=pso[:, 8:9])
            nc.vector.tensor_scalar_mul(olT_t, pso[:, 0:8], rs_t)

            # --- broadcast along y: single DVE copy with stride-0 read on y
            T3 = T_t.rearrange("p (c y) -> p c y", y=64)
            ola = olT_t[:]
            ol_b = bass.AP(
                tensor=ola.tensor, offset=ola.offset,
                ap=[list(ola.ap[0]), list(ola.ap[1]), [0, 64]],
            )
            nc.vector.tensor_copy(out=T3, in_=ol_b)

            # --- store: out[c, x, y] <- T[x, c, y], split across both rings
            out_ap = out_d[:].rearrange("c (x y) -> c x y", y=64).transpose([1, 0, 2])
            nc.sync.dma_start(out=out_ap[0:32], in_=T3[0:32])
            nc.scalar.dma_start(out=out_ap[32:64], in_=T3[32:64])

    nc.finalize()
    return nc


def _get_nc():
    if "nc" not in _CACHE:
        _CACHE["nc"] = _build_nc()
    return _CACHE["nc"]


def kernel(**inputs):
    global LAST_RESULTS
    from concourse.bass_utils import run_bass_kernel_spmd

    f = np.ascontiguousarray(inputs["f"], np.float32)
    w_qkv = np.ascontiguousarray(inputs["w_qkv"], np.float32)[:, :, 0, 0]  # [192,64]
    wq = np.ascontiguousarray(inputs["wq"], np.float32)
    wk = np.ascontiguousarray(inputs["wk"], np.float32)
    bq = np.ascontiguousarray(inputs["bq"], np.float32)
    bk = np.ascontiguousarray(inputs["bk"], np.float32)

    fpad = np.zeros((64, 68, 68), np.float32)
    fpad[:, 2:66, 2:66] = f[0]
    fr = fpad[:, ORDER, :]
    frs = np.zeros_like(fr)
    frs[:, :, 0:67] = fpad[:, ORDER, 1:68]
    f_dram = np.concatenate([fr, frs], axis=0).reshape(128, 67 * 68).astype(np.float16)

    W1q, W1k, Wv = w_qkv[0:64], w_qkv[64:128], w_qkv[128:192]
    in_maps = []
    for i in range(N_CORES):
        sl = slice(8 * i, 8 * i + 8)
        weq = np.einsum('oiyx,id->dyxo', wq[sl], W1q)  # [64,11,11,8]
        wek = np.einsum('oiyx,id->dyxo', wk[sl], W1k)
        wst = np.zeros((128, W_COLS), np.float32)
        for n, (ky, p, _r0, _kx0) in enumerate(MM_ARGS):
            c0 = 16 * n
            wst[0:64, c0:c0 + 8] = weq[:, ky, 2 * p, :]
            wst[0:64, c0 + 8:c0 + 16] = wek[:, ky, 2 * p, :]
            if 2 * p + 1 <= 10:
                wst[64:128, c0:c0 + 8] = weq[:, ky, 2 * p + 1, :]
                wst[64:128, c0 + 8:c0 + 16] = wek[:, ky, 2 * p + 1, :]
        wst[0:64, C_WVT:C_WVT + 8] = Wv[sl].T
        wst[0:8, C_BIAS] = bq[sl]
        wst[8:16, C_BIAS] = bk[sl]
        for c in range(8):
            wst[8 + c, C_SEL + c] = 1.0  # k-half selection matrix
        in_maps.append({"f": f_dram, "w": wst.astype(np.float16)})

    nc = _get_nc()
    res = run_bass_kernel_spmd(nc, in_maps, core_ids=list(range(N_CORES)))
    LAST_RESULTS = res
    out = np.concatenate([r["out"] for r in res.results], axis=0)  # [64, 4096]
    return out.reshape(1, 64, 64, 64)


# revision 13
# speedup vs baseline: 1.1747x; 1.1747x over previous
"""Trainium2 Bass kernel for nn_Attention_32049045963483 (sparse_attention).

Math collapse (validated vs reference at ~4e-4 rel err):
  - qkv 1x1 conv folds into the 11x11/stride-8 down-convs (host-side
    weight composition): w_eff[d, ky, kx, oc] = sum_ic wq[oc,ic,ky,kx] W1[ic,d]
  - nearest-neighbor 64x upsample + softmax == softmax of the low-res map;
    out row I depends only on x = I//64.
  - v enters only through 64-wide block sums: vbar = Wv @ fbar,
    fbar[d,x] = sum_y f[d,x,y] (v never materializes).
  - out[c,x,y] = (sum_j e[j,x] vbar[j,c]) / (64 sum_j e[j,x]), broadcast on y.

Kernel structure (one head per core, 8 cores):
  - DIRECT conv: 66 matmuls accumulate into one PSUM [16,64] tile
    (q oc 0..7, k oc 8..15).  K = 128 = (64 d) x (2 kx taps): partitions
    64..127 hold f shifted by one column, so kx pairs (0,1)..(8,9) use the
    full PE contraction; kx=10 has zero weights in the upper half.
  - f rows are CLASS-ORDERED (r mod 8) in DRAM so arriving chunks unlock
    their matmuls: class c serves ky = c (and ky = c+8 for c < 3).
  - conv stationaries are packed in CONSUMPTION order; the first 24
    (classes 0-1) split across both HWDGE rings ahead of f; the rest on
    the gpsimd SWDGE queue.
  - gelu via tanh-approx (exp_and_others act table has Tanh+Exp+Square:
    NO mid-kernel activation-table reload), fused to 4 ops:
      x = psqk+bias (DVE) || u = (psqk+bias)^2 (Act Square)
      v2 = (u + a/b)*x (DVE stt);  th = tanh(b*v2) (Act scale)
      qk = (th + 1)*x (DVE stt) = 2*gelu(x); the 4x folds into exp scale.
  - fbar per class on DVE (overlaps conv), vbar via one small matmul.
  - output: x-major broadcast then transposed DMA split over both rings.
"""

import numpy as np

N_CORES = 8
SCALE = 8.0 ** -0.5  # dim_head ** -0.5

ORDER = [r for c in range(8) for r in range(c, 67, 8)]
CLS_OFF = [0, 9, 18, 27, 35, 43, 51, 59]
# conv matmul emission order: (class, ky, kx-pair); stationary block j is
# packed at w columns 16j
MM_ARGS = []
for _c in range(8):
    for _ky in ([_c, _c + 8] if _c + 8 <= 10 else [_c]):
        _pos0 = 0 if _ky == _c else 1
        for _p in range(6):
            MM_ARGS.append((_ky, _p, CLS_OFF[_c] + _pos0, 2 * _p))
N_MM = len(MM_ARGS)  # 66
C_WVT = 16 * N_MM            # 1056
C_BIAS = C_WVT + 8           # 1064
C_SEL = C_BIAS + 1           # 1065
W_COLS = C_SEL + 8           # 1073
W1_COLS = 16 * 24            # stationaries for classes 0 and 1 (24 matmuls)

_CACHE = {}
LAST_RESULTS = None  # BassKernelResults of the most recent run (for test harness)


def _dep(after, before, sync=False):
    from concourse.tile import add_dep_helper

    a = getattr(after, "ins", after)
    b = getattr(before, "ins", before)
    add_dep_helper(a, b, sync=sync, reason="pin order")


def _build_nc():
    from contextlib import ExitStack

    import concourse.bacc as bacc
    import concourse.bass as bass
    import concourse.mybir as mybir
    import concourse.tile as tile

    f32 = mybir.dt.float32
    f16 = mybir.dt.float16
    X = mybir.AxisListType.X
    AF = mybir.ActivationFunctionType
    OP = mybir.AluOpType

    nc = bacc.Bacc("TRN2", target_bir_lowering=False)

    f_d = nc.dram_tensor("f", [128, 67 * 68], f16, kind="ExternalInput")
    w_d = nc.dram_tensor("w", [128, W_COLS], f16, kind="ExternalInput")
    out_d = nc.dram_tensor("out", [8, 4096], f32, kind="ExternalOutput")

    with tile.TileContext(nc) as tc:
        with ExitStack() as ctx:
            sb = ctx.enter_context(tc.tile_pool(name="sb", bufs=1))
            ps = ctx.enter_context(tc.tile_pool(name="ps", bufs=1, space="PSUM"))

            f_t = sb.tile([128, 67 * 68], f16)
            w_t = sb.tile([128, W_COLS], f16)
            x_t = sb.tile([16, 64], f32)
            u_t = sb.tile([16, 64], f32)
            v2_t = sb.tile([16, 64], f32)
            th_t = sb.tile([16, 64], f32)
            qk_t = sb.tile([16, 64], f16)
            k_t = sb.tile([8, 64], f16)
            e_t = sb.tile([64, 64], f16)
            fbar_t = sb.tile([64, 64], f16)
            vaug_t = sb.tile([64, 9], f16)
            bias_t = sb.tile([16, 1], f32)
            rs_t = sb.tile([64, 1], f32)
            olT_t = sb.tile([64, 8], f32)
            T_t = sb.tile([64, 8 * 64], f32)
            scr_t = sb.tile([1, 1], f32)
            scr2_t = sb.tile([1, 1], f32)

            # --- input DMAs: first conv stationaries (classes 0-1) split
            # over both HWDGE rings, then f classes; bulk weights on SWDGE.
            def fseg(eng, c0, c1):
                a = CLS_OFF[c0] * 68
                b = (CLS_OFF[c1] + (9 if c1 < 3 else 8)) * 68
                return eng.dma_start(out=f_t[:, a:b], in_=f_d[:, a:b])

            d_w = nc.sync.dma_start(out=w_t[:], in_=w_d[:])
            fseg(nc.scalar, 0, 0)
            fseg(nc.sync, 2, 3)
            fseg(nc.scalar, 1, 1)
            d_c45 = fseg(nc.scalar, 4, 5)
            fseg(nc.sync, 6, 7)
            d_w2 = d_w

            # constants + act-table preload (exp_and_others: Exp/Tanh/Square/
            # Copy in one set -> single ACT_TABLE_LOAD, after scalar triggers)
            nc.vector.memset(scr_t, 0.0)
            nc.vector.memset(vaug_t[:, 8:9], 64.0)
            de = nc.scalar.activation(out=scr2_t, in_=scr_t, func=AF.Exp)
            _dep(de, d_c45)
            bcp = nc.vector.tensor_copy(out=bias_t, in_=w_t[0:16, C_BIAS:C_BIAS + 1])
            _dep(bcp, d_w2)

            # --- direct conv: 66 matmuls accumulate into psqk [16, 64]
            f3 = f_t.rearrange("p (r c) -> p r c", c=68)
            psqk = ps.tile([16, 64], f32, tag="A")
            for n, (_ky, _p, r0, kx0) in enumerate(MM_ARGS):
                nc.tensor.matmul(
                    psqk, w_t[:, 16 * n:16 * n + 16],
                    f3[:, r0:r0 + 8, kx0:kx0 + 57:8],
                    start=(n == 0), stop=(n == N_MM - 1),
                )

            # --- fbar per class on DVE (runs under the conv).  fp16 out is
            # fine: 64-term sums of O(1) values, 40x margin under the gate.
            with nc.allow_low_precision("fbar fp16 is within tolerance"):
                for c in range(8):
                    p0 = 1 if c <= 1 else 0
                    x0 = (c - 2) % 8
                    nc.vector.tensor_reduce(
                        out=fbar_t[:, x0:x0 + 57:8],
                        in_=f3[0:64, CLS_OFF[c] + p0:CLS_OFF[c] + p0 + 8, 2:66],
                        axis=X, op=OP.add,
                    )

            # --- vbar: psv[x, c] = sum_d fbar[d, x] wvt[d, c]  (during gelu)
            psv = ps.tile([64, 8], f32, tag="C")
            nc.tensor.matmul(
                psv, fbar_t, w_t[0:64, C_WVT:C_WVT + 8], start=True, stop=True
            )
            nc.vector.tensor_copy(out=vaug_t[:, 0:8], in_=psv)

            # --- gelu (tanh approx): qk = 2*gelu(psqk + bias)
            #   x = psqk+bias;  u = x^2;  v2 = (u + a/b)*x;  th = tanh(b*v2)
            #   qk = (th+1)*x
            A_GELU = 0.79788456
            B_GELU = 0.0356774
            nc.vector.tensor_scalar_add(x_t, psqk, bias_t)
            nc.scalar.activation(out=u_t, in_=psqk, func=AF.Square, bias=bias_t)
            nc.vector.scalar_tensor_tensor(
                v2_t, u_t, A_GELU / B_GELU, x_t, op0=OP.add, op1=OP.mult
            )
            nc.scalar.activation(out=th_t, in_=v2_t, func=AF.Tanh, scale=B_GELU)
            nc.vector.scalar_tensor_tensor(
                qk_t, th_t, 1.0, x_t, op0=OP.add, op1=OP.mult
            )

            # k half -> base-0 tile via PE selection matmul (engines cannot
            # read at a partition offset), then DVE copy PSUM -> SBUF
            psk = ps.tile([8, 64], f32, tag="C")
            nc.tensor.matmul(psk, w_t[0:16, C_SEL:C_SEL + 8], qk_t,
                             start=True, stop=True)
            nc.vector.tensor_copy(out=k_t, in_=psk)

            # --- dots^T[j, i] = sum_c k[c, j] q[c, i];  e = exp(scale/4 * .)
            psd = ps.tile([64, 64], f32, tag="B")
            nc.tensor.matmul(psd, k_t, qk_t[0:8, :], start=True, stop=True)
            nc.scalar.activation(out=e_t, in_=psd, func=AF.Exp, scale=SCALE / 4.0)

            # --- out_u[x, 0:8] = sum_j e[j, x] vbar[j, c]; col 8 = 64*sum_j e
            pso = ps.tile([64, 9], f32, tag="D")
            nc.tensor.matmul(pso, e_t, vaug_t, start=True, stop=True)
            nc.vector.reciprocal(out=rs_t, in_=pso[:, 8:9])
            nc.vector.tensor_scalar_mul(olT_t, pso[:, 0:8], rs_t)

            # --- broadcast along y: single DVE copy with stride-0 read on y
            T3 = T_t.rearrange("p (c y) -> p c y", y=64)
            ola = olT_t[:]
            ol_b = bass.AP(
                tensor=ola.tensor, offset=ola.offset,
                ap=[list(ola.ap[0]), list(ola.ap[1]), [0, 64]],
            )
            nc.vector.tensor_copy(out=T3, in_=ol_b)

            # --- store: out[c, x, y] <- T[x, c, y], split across both rings
            out_ap = out_d[:].rearrange("c (x y) -> c x y", y=64).transpose([1, 0, 2])
            nc.sync.dma_start(out=out_ap[0:32], in_=T3[0:32])
            nc.scalar.dma_start(out=out_ap[32:64], in_=T3[32:64])

    nc.finalize()
    return nc


def _get_nc():
    if "nc" not in _CACHE:
        _CACHE["nc"] = _build_nc()
    return _CACHE["nc"]


def kernel(**inputs):
    global LAST_RESULTS
    from concourse.bass_utils import run_bass_kernel_spmd

    f = np.ascontiguousarray(inputs["f"], np.float32)
    w_qkv = np.ascontiguousarray(inputs["w_qkv"], np.float32)[:, :, 0, 0]  # [192,64]
    wq = np.ascontiguousarray(inputs["wq"], np.float32)
    wk = np.ascontiguousarray(inputs["wk"], np.float32)
    bq = np.ascontiguousarray(inputs["bq"], np.float32)
    bk = np.ascontiguousarray(inputs["bk"], np.float32)

    fpad = np.zeros((64, 68, 68), np.float32)
    fpad[:, 2:66, 2:66] = f[0]
    fr = fpad[:, ORDER, :]
    frs = np.zeros_like(fr)
    frs[:, :, 0:67] = fpad[:, ORDER, 1:68]
    f_dram = np.concatenate([fr, frs], axis=0).reshape(128, 67 * 68).astype(np.float16)

    W1q, W1k, Wv = w_qkv[0:64], w_qkv[64:128], w_qkv[128:192]
    in_maps = []
    for i in range(N_CORES):
        sl = slice(8 * i, 8 * i + 8)
        weq = np.einsum('oiyx,id->dyxo', wq[sl], W1q)  # [64,11,11,8]
        wek = np.einsum('oiyx,id->dyxo', wk[sl], W1k)
        wst = np.zeros((128, W_COLS), np.float32)
        for n, (ky, p, _r0, _kx0) in enumerate(MM_ARGS):
            c0 = 16 * n
            wst[0:64, c0:c0 + 8] = weq[:, ky, 2 * p, :]
            wst[0:64, c0 + 8:c0 + 16] = wek[:, ky, 2 * p, :]
            if 2 * p + 1 <= 10:
                wst[64:128, c0:c0 + 8] = weq[:, ky, 2 * p + 1, :]
                wst[64:128, c0 + 8:c0 + 16] = wek[:, ky, 2 * p + 1, :]
        wst[0:64, C_WVT:C_WVT + 8] = Wv[sl].T
        wst[0:8, C_BIAS] = bq[sl]
        wst[8:16, C_BIAS] = bk[sl]
        for c in range(8):
            wst[8 + c, C_SEL + c] = 1.0  # k-half selection matrix
        in_maps.append({"f": f_dram, "w": wst.astype(np.float16)})

    nc = _get_nc()
    res = run_bass_kernel_spmd(nc, in_maps, core_ids=list(range(N_CORES)))
    LAST_RESULTS = res
    out = np.concatenate([r["out"] for r in res.results], axis=0)  # [64, 4096]
    return out.reshape(1, 64, 64, 64)
